# revision 10
# baseline (speedup 1.0000x reference)
"""3-layer GAT (GATNet) on 8 Trainium2 NeuronCores via Bass/Tile.

Sharding: nodes (and their incident edges, grouped by destination) are
partitioned across the 8 cores; weights are replicated. Per layer:
  dense:  H_ext = A @ W_ext for this core's node shard.  W_ext carries
          extra columns so the same matmul also produces the attention
          scores s_src/s_dst per node.  Rows are written to a gather
          table as [h | 1.0 | s_src | pad] (row bytes % 256 == 0).
  AllGather of the gather table across the 8 cores.
  agg:    per 128-destination block, dma_gather the source-node rows of
          the block's edges (two gathers because dma_gather indices are
          int16: table split at row 32768), fetch s_dst per edge with a
          small indirect DMA from the core-local s_dst array, compute
          w = exp(leaky_relu(s_src+s_dst)), build per-tile selection
          matrices Se_w[j,d] = (dst_local[j]==d)*w[j] with one fused DVE
          tensor_scalar against an iota row, and accumulate
          PSUM[d, :] += Se_w^T @ G on the PE.  The constant-1 column in
          each row makes the same matmuls produce the softmax
          denominators.  Epilogue: divide, add bias, leaky_relu, and
          PE-transpose into the next layer's lhsT chunks (kept in SBUF).
Layer 3 aggregates only for the output nodes (first node of each graph,
batch = arange // graph_size), so it is nearly free.
"""

import os
import sys

import numpy as np

sys.path.insert(0, "/opt/trn_rl_repo")

from contextlib import ExitStack  # noqa: E402

from concourse import bacc, bass, mybir, tile  # noqa: E402
from concourse.bass_utils import run_bass_kernel_spmd  # noqa: E402
from concourse.masks import make_identity  # noqa: E402

F32 = mybir.dt.float32
BF16 = mybir.dt.bfloat16
I16 = mybir.dt.int16
I32 = mybir.dt.int32

P = 128
NCORES = 8
NEG_ATT = 0.2
NEG_ACT = 0.01

# timing-attribution knobs (break correctness; timing only)
SKIP_COLL = os.environ.get("GAT_SKIP_COLL", "0") == "1"
SKIP_GS = os.environ.get("GAT_SKIP_GS", "0") == "1"
SKIP_G = os.environ.get("GAT_SKIP_G", "0") == "1"


def _ceil(a, b):
    return -(-a // b)


class Cfg:
    def __init__(self, n_nodes=50000, in_ch=128, hid=256, out_ch=16, graph=50,
                 use_bf16_tab=False):
        assert in_ch == 128
        self.n_nodes = n_nodes
        self.in_ch = in_ch
        self.hid = hid
        self.out_ch = out_ch
        self.graph = graph
        self.use_bf16_tab = use_bf16_tab
        self.nb = _ceil(_ceil(n_nodes, NCORES), P)   # dst blocks per core
        self.rows = self.nb * P                      # padded rows per core
        self.ntot = self.rows * NCORES
        # gather-table row length in elements; row bytes must be % 256 == 0
        self.row12 = 384 if use_bf16_tab else 320  # 768 B / 1280 B
        self.row3 = 64            # f32: 256 B
        self.tdt = BF16 if use_bf16_tab else F32
        # dma_gather indices are int16; split the gather table so both
        # halves stay addressable
        self.split = min(32768, ((self.ntot // 2) // P) * P)
        assert self.split <= 32767 + 1 and self.ntot - self.split <= 32768


# ----------------------------------------------------------------------------
# host-side edge preprocessing
# ----------------------------------------------------------------------------

def preprocess(edge_src, edge_dst, cfg: Cfg):
    """Build the per-core edge-stream arrays.

    Edge slot layout per (core, dst-block): region A (src < SPLIT) slots
    [0, TA*128), region B slots [TA*128, (TA+TB)*128).  Slot s maps to
    gathered-tile position (partition s%128, tile s//128).  Padding slots
    use gather index 0 (a valid row) and dst_local 999 (never matches)."""
    src = np.asarray(edge_src).astype(np.int64)
    dst = np.asarray(edge_dst).astype(np.int64)
    R, NB = cfg.rows, cfg.nb
    N = cfg.n_nodes

    core = dst // R
    blk = (dst - core * R) // P
    SPLIT = cfg.split
    reg = (src >= SPLIT).astype(np.int64)

    # ---- layers 1/2 stream: group edges by (core, block, region) ----
    key = (core * NB + blk) * 2 + reg
    order = np.argsort(key, kind="stable")
    ks, ss, ds = key[order], src[order], dst[order]
    starts = np.searchsorted(ks, np.arange(2 * NCORES * NB))
    pos = np.arange(len(ks)) - starts[ks]

    nA = np.zeros((NCORES, NB), np.int64)
    nB = np.zeros((NCORES, NB), np.int64)
    np.add.at(nA, (core, blk), 1 - reg)
    np.add.at(nB, (core, blk), reg)
    TA = max(1, _ceil(int(nA.max()), P))
    TB = max(1, _ceil(int(nB.max()), P))
    T = TA + TB

    c_s = ks // (2 * NB)
    b_s = (ks // 2) % NB
    r_s = ks % 2
    slot = np.where(r_s == 0, pos, TA * P + pos)
    p_s = slot % P
    t_s = slot // P

    gidx16 = np.zeros((NCORES, NB, 16, T * 8), np.int16)
    dloc16 = np.zeros((NCORES, NB, 16, T * 8), np.int16)
    dstcol = np.full((NCORES, NB, P, T), 999.0, np.float32)

    wcol = np.where(r_s == 0, pos // 16, TA * 8 + pos // 16)
    val = np.where(r_s == 0, ss, ss - SPLIT).astype(np.int16)
    gidx16[c_s, b_s, pos % 16, wcol] = val

    rloc = ds - c_s * R
    # slot order for the s_dst gather uses the same wrapped layout, but
    # slot index here runs over the full T*128 slot space
    wcol_d = np.where(r_s == 0, slot // 16, slot // 16)
    dloc16[c_s, b_s, slot % 16, wcol_d] = rloc.astype(np.int16)
    dstcol[c_s, b_s, p_s, t_s] = (rloc % P).astype(np.float32)

    gidx = np.tile(gidx16, (1, 1, 8, 1))  # replicate across the 8 Q7 groups
    dloc = np.tile(dloc16, (1, 1, 8, 1))
    # pack [gidx | dloc | dstcol-as-i16] so one DMA per block loads all
    epack = np.concatenate(
        [gidx, dloc, dstcol.view(np.int16)], axis=-1)

    # ---- layer-3 stream: only edges into the zero-mask (output) nodes ----
    nodes = np.arange(N)
    zmask_node = (nodes % cfg.graph) == 0
    zcounts = np.bincount(nodes[zmask_node] // R, minlength=NCORES)
    zslot_of = np.full(N, -1, np.int64)
    for c in range(NCORES):
        zn = nodes[zmask_node & (nodes // R == c)]
        zslot_of[zn] = np.arange(len(zn))

    sel = zmask_node[dst]
    s3, d3 = src[sel], dst[sel]
    c3 = d3 // R
    r3 = (s3 >= SPLIT).astype(np.int64)
    key3 = c3 * 2 + r3
    o3 = np.argsort(key3, kind="stable")
    k3, s3, d3 = key3[o3], s3[o3], d3[o3]
    starts3 = np.searchsorted(k3, np.arange(2 * NCORES))
    pos3 = np.arange(len(k3)) - starts3[k3]
    n3A = np.zeros(NCORES, np.int64)
    n3B = np.zeros(NCORES, np.int64)
    np.add.at(n3A, c3, 1 - r3)
    np.add.at(n3B, c3, r3)
    T3A = max(1, _ceil(int(n3A.max()), P))
    T3B = max(1, _ceil(int(n3B.max()), P))
    T3 = T3A + T3B

    cc3 = k3 // 2
    rr3 = k3 % 2
    slot3 = np.where(rr3 == 0, pos3, T3A * P + pos3)
    p3 = slot3 % P
    t3 = slot3 // P
    gidx316 = np.zeros((NCORES, 16, T3 * 8), np.int16)
    dloc316 = np.zeros((NCORES, 16, T3 * 8), np.int16)
    dstcol3 = np.full((NCORES, P, T3), 999.0, np.float32)
    wcol3 = np.where(rr3 == 0, pos3 // 16, T3A * 8 + pos3 // 16)
    val3 = np.where(rr3 == 0, s3, s3 - SPLIT).astype(np.int16)
    gidx316[cc3, pos3 % 16, wcol3] = val3
    rl3 = d3 - cc3 * R
    dloc316[cc3, slot3 % 16, slot3 // 16] = rl3.astype(np.int16)
    dstcol3[cc3, p3, t3] = zslot_of[d3].astype(np.float32)
    gidx3 = np.tile(gidx316, (1, 8, 1))
    dloc3 = np.tile(dloc316, (1, 8, 1))
    epack3 = np.concatenate(
        [gidx3, dloc3, dstcol3.view(np.int16)], axis=-1)

    return dict(TA=TA, TB=TB, T3A=T3A, T3B=T3B,
                epack=epack, epack3=epack3,
                zcounts=zcounts)


# ----------------------------------------------------------------------------
# program builder
# ----------------------------------------------------------------------------

def build_program(cfg: Cfg, TA, TB, T3A, T3B, repeat=1):
    NB, R = cfg.nb, cfg.rows
    T, T3 = TA + TB, T3A + T3B
    ROW, ROW3 = cfg.row12, cfg.row3
    tdt = cfg.tdt
    HID, OUT = cfg.hid, cfg.out_ch
    NTOT = cfg.ntot
    SPLIT = cfg.split

    nc = bacc.Bacc("TRN2", target_bir_lowering=False, debug=False,
                   num_devices=NCORES)

    # ---- I/O ----
    xT = nc.dram_tensor("xT", [P, R], F32, kind="ExternalInput")
    W1e = nc.dram_tensor("W1e", [P, HID + 3], F32, kind="ExternalInput")
    W2e = nc.dram_tensor("W2e", [HID, HID + 3], F32, kind="ExternalInput")
    W3e = nc.dram_tensor("W3e", [HID, OUT + 3], F32, kind="ExternalInput")
    b1 = nc.dram_tensor("b1", [1, HID], F32, kind="ExternalInput")
    b2 = nc.dram_tensor("b2", [1, HID], F32, kind="ExternalInput")
    b3 = nc.dram_tensor("b3", [1, OUT], F32, kind="ExternalInput")
    epack = nc.dram_tensor("epack", [NB, P, T * 18], I16,
                           kind="ExternalInput")
    epack3 = nc.dram_tensor("epack3", [P, T3 * 18], I16,
                            kind="ExternalInput")
    out_d = nc.dram_tensor("out", [P, OUT], F32, kind="ExternalOutput")

    # ---- internal DRAM ----
    h1shard = nc.dram_tensor("h1shard", [R, ROW], tdt)
    h1tab = nc.dram_tensor("h1tab", [NTOT, ROW], tdt, addr_space="Shared")
    h2shard = nc.dram_tensor("h2shard", [R, ROW], tdt)
    h2tab = nc.dram_tensor("h2tab", [NTOT, ROW], tdt, addr_space="Shared")
    h3shard = nc.dram_tensor("h3shard", [R, ROW3], F32)
    h3tab = nc.dram_tensor("h3tab", [NTOT, ROW3], F32, addr_space="Shared")
    SROW = 64
    stab = [nc.dram_tensor(f"stab{i}", [R, SROW], F32) for i in (1, 2, 3)]

    rg = [list(range(NCORES))]

    # persistent next-layer lhsT chunks (A^T), reused across layer pairs
    aT = [nc.alloc_sbuf_tensor("aT0", [P, R], F32),
          nc.alloc_sbuf_tensor("aT1", [P, R], F32)]

    with tile.TileContext(nc) as tc, ExitStack() as ctx:
        cpool = ctx.enter_context(tc.tile_pool(name="const", bufs=1))
        wpool = ctx.enter_context(tc.tile_pool(name="weights", bufs=1))
        lt_pool = ctx.enter_context(tc.tile_pool(name="lhsT", bufs=3))
        row_pool = ctx.enter_context(tc.tile_pool(name="rows", bufs=3))
        idx_pool = ctx.enter_context(tc.tile_pool(name="idx", bufs=4))
        g_pool = ctx.enter_context(tc.tile_pool(name="gather", bufs=2))
        s_pool = ctx.enter_context(tc.tile_pool(name="scal", bufs=4))
        se_pool = ctx.enter_context(tc.tile_pool(name="sew", bufs=3))
        a_pool = ctx.enter_context(tc.tile_pool(name="arow", bufs=3))
        ps_dense = ctx.enter_context(
            tc.tile_pool(name="psd", bufs=2, space="PSUM"))
        ps_agg = ctx.enter_context(
            tc.tile_pool(name="psa", bufs=3, space="PSUM"))
        ps_tp = ctx.enter_context(
            tc.tile_pool(name="pst", bufs=1, space="PSUM"))
        ps_bc = ctx.enter_context(
            tc.tile_pool(name="psb", bufs=1, space="PSUM"))

        # constants
        ident = cpool.tile([P, P], F32, tag="ident")
        make_identity(nc, ident[:])
        TMAX = max(T, T3)
        iota_i = cpool.tile([P, TMAX * P], I32, tag="iotai")
        nc.gpsimd.iota(iota_i[:], pattern=[[0, TMAX], [1, P]], base=0,
                       channel_multiplier=0)
        iota_f = cpool.tile([P, TMAX * P], F32, tag="iotaf")
        nc.vector.tensor_copy(out=iota_f[:], in_=iota_i[:])
        ones1 = cpool.tile([1, P], F32, tag="ones1")
        nc.vector.memset(ones1[:], 1.0)

        # preload weights
        w1_sb = wpool.tile([P, HID + 3], F32, tag="w1")
        nc.sync.dma_start(out=w1_sb[:], in_=W1e[:, :])
        w2_sb = [wpool.tile([P, HID + 3], F32, tag=f"w2_{k}",
                            name=f"w2sb{k}") for k in range(2)]
        for k in range(2):
            nc.sync.dma_start(out=w2_sb[k][:], in_=W2e[k * P:(k + 1) * P, :])
        w3_sb = [wpool.tile([P, OUT + 3], F32, tag=f"w3_{k}",
                            name=f"w3sb{k}") for k in range(2)]
        for k in range(2):
            nc.sync.dma_start(out=w3_sb[k][:], in_=W3e[k * P:(k + 1) * P, :])

        def bias_bcast(bd, C, tag):
            brow = cpool.tile([1, C], F32, tag=f"brow_{tag}")
            nc.sync.dma_start(out=brow[:], in_=bd[:, :])
            bps = ps_bc.tile([P, C], F32, tag="bps")
            nc.tensor.matmul(bps[:], lhsT=ones1[:], rhs=brow[:],
                             start=True, stop=True)
            bbc = cpool.tile([P, C], F32, tag=f"bbc_{tag}")
            nc.vector.tensor_copy(out=bbc[:], in_=bps[:])
            return bbc

        # ------------------------------------------------------------------
        def dense_phase(layer, w_tiles, shard, sdacc_dst, rowlen, row_dt,
                        one_col, sd_col):
            """H_ext = A @ W_ext for this core's rows; writes the gather
            table rows [h | 1 | s_src | pad] + the local s_dst array."""
            n_extcols = sd_col + 1
            for it in range(NB):
                if layer == 1:
                    lt = lt_pool.tile([P, P], F32, tag="xT")
                    nc.sync.dma_start(out=lt[:],
                                      in_=xT[:, it * P:(it + 1) * P])
                    lts = [lt[:]]
                else:
                    lts = [aT[k].ap()[:, it * P:(it + 1) * P]
                           for k in range(2)]
                ps = ps_dense.tile([P, n_extcols], F32, tag="dps")
                for k, lt_ap in enumerate(lts):
                    nc.tensor.matmul(ps[:], lhsT=lt_ap, rhs=w_tiles[k][:],
                                     start=(k == 0), stop=(k == len(lts) - 1))
                row = row_pool.tile([P, rowlen], row_dt, tag=f"row{rowlen}")
                if row_dt == BF16:
                    # [h bf16 | 1.0 | s_src_hi | s_src_lo | pad]
                    nc.vector.tensor_copy(out=row[:, 0:one_col],
                                          in_=ps[:, 0:one_col])
                    nc.vector.memset(row[:, one_col:rowlen], 0.0)
                    nc.vector.memset(row[:, one_col:one_col + 1], 1.0)
                    hc = one_col + 1
                    nc.vector.tensor_copy(out=row[:, hc:hc + 1],
                                          in_=ps[:, hc:hc + 1])
                    shi = s_pool.tile([P, 1], F32, tag="shi")
                    nc.vector.tensor_copy(out=shi[:], in_=row[:, hc:hc + 1])
                    slo = s_pool.tile([P, 1], F32, tag="slo")
                    nc.vector.tensor_tensor(out=slo[:], in0=ps[:, hc:hc + 1],
                                            in1=shi[:],
                                            op=mybir.AluOpType.subtract)
                    nc.vector.tensor_copy(out=row[:, hc + 1:hc + 2],
                                          in_=slo[:])
                else:
                    # psum col one_col is 0 (zero column in W_ext);
                    # col one_col+1 is s_src
                    nc.vector.tensor_copy(out=row[:, 0:one_col + 2],
                                          in_=ps[:, 0:one_col + 2])
                    nc.vector.memset(row[:, one_col:one_col + 1], 1.0)
                    nc.vector.memset(row[:, one_col + 2:rowlen], 0.0)
                srow = row_pool.tile([P, SROW], F32, tag="srow")
                nc.vector.memset(srow[:, 1:SROW], 0.0)
                nc.vector.tensor_copy(out=srow[:, 0:1],
                                      in_=ps[:, sd_col:sd_col + 1])
                nc.sync.dma_start(out=shard[it * P:(it + 1) * P, :],
                                  in_=row[:])
                nc.sync.dma_start(
                    out=sdacc_dst[it * P:(it + 1) * P, :], in_=srow[:])

        # ------------------------------------------------------------------
        def agg_phase(layer, tab, sd_t, nblocks, tA, tB, rowlen, row_dt,
                      epack_t, C_out, bbc):
            tT = tA + tB
            n_mm = C_out + 1  # h columns + the constant-1 (denominator) col
            tabA = tab[0:SPLIT, :]
            tabB = tab[SPLIT:NTOT, :]
            G_skip = None
            if SKIP_G:
                G_skip = cpool.tile([P, tT * rowlen], row_dt,
                                    tag=f"Gskip{tT}x{rowlen}")
                nc.vector.memset(G_skip[:], 0.001)
            for b in range(nblocks):
                ep = idx_pool.tile([P, tT * 18], I16, tag="ep")
                if nblocks == 1:
                    nc.sync.dma_start(out=ep[:], in_=epack_t[:, :])
                else:
                    nc.sync.dma_start(out=ep[:], in_=epack_t[b, :, :])
                gi = ep[:, 0:tT * 8]
                dl = ep[:, tT * 8:tT * 16]
                dc = ep[:, tT * 16:tT * 18].bitcast(F32)
                if SKIP_G:
                    G = G_skip
                else:
                    G = g_pool.tile([P, tT * rowlen], row_dt,
                                    tag=f"G{rowlen}")
                G3d = G[:].rearrange("p (t c) -> p t c", c=rowlen)
                if not SKIP_G:
                    nc.gpsimd.dma_gather(
                        out_ap=G3d[:, 0:tA, :], in_ap=tabA,
                        idxs_ap=gi[:, 0:tA * 8], num_idxs=tA * P,
                        num_idxs_reg=tA * P, elem_size=rowlen,
                        elem_step=rowlen, single_packet=False)
                    nc.gpsimd.dma_gather(
                        out_ap=G3d[:, tA:tT, :], in_ap=tabB,
                        idxs_ap=gi[:, tA * 8:tT * 8], num_idxs=tB * P,
                        num_idxs_reg=tB * P, elem_size=rowlen,
                        elem_step=rowlen, single_packet=False)
                if not SKIP_GS:
                    Gs = g_pool.tile([P, tT * SROW], F32, tag="Gs")
                    Gs3d = Gs[:].rearrange("p (t c) -> p t c", c=SROW)
                    nc.gpsimd.dma_gather(
                        out_ap=Gs3d[:, :, :], in_ap=sd_t[:, :],
                        idxs_ap=dl, num_idxs=tT * P,
                        num_idxs_reg=tT * P, elem_size=SROW, elem_step=SROW,
                        single_packet=False)
                    sdp = Gs[:, 0::SROW]
                else:
                    sdp = None
                if row_dt == BF16:
                    sfull = s_pool.tile([P, tT], F32, tag="sfull")
                    nc.vector.tensor_tensor(
                        out=sfull[:], in0=G[:, C_out + 1::rowlen],
                        in1=G[:, C_out + 2::rowlen], op=mybir.AluOpType.add)
                    ssrc = sfull[:]
                else:
                    ssrc = G[:, C_out + 1::rowlen]
                z = s_pool.tile([P, tT], F32, tag="z")
                if sdp is None:
                    nc.vector.tensor_copy(out=z[:], in_=ssrc)
                else:
                    nc.vector.tensor_tensor(out=z[:], in0=ssrc, in1=sdp,
                                            op=mybir.AluOpType.add)
                e = s_pool.tile([P, tT], F32, tag="e")
                nc.vector.scalar_tensor_tensor(
                    out=e[:], in0=z[:], scalar=NEG_ATT, in1=z[:],
                    op0=mybir.AluOpType.mult, op1=mybir.AluOpType.max)
                w = s_pool.tile([P, tT], F32, tag="w")
                nc.scalar.activation(w[:], e[:],
                                     mybir.ActivationFunctionType.Exp)
                ps = ps_agg.tile([P, n_mm], F32, tag="aps")
                swa = se_pool.tile([P, tT * P], row_dt, tag="swa")
                dc3 = dc.unsqueeze(-1).to_broadcast([P, tT, P])
                w3 = w[:].unsqueeze(-1).to_broadcast([P, tT, P])
                swa3 = swa[:].rearrange("p (t d) -> p t d", d=P)
                nc.vector.tensor_tensor(
                    out=swa3, in0=iota_f[:, 0:tT * P].rearrange(
                        "p (t d) -> p t d", d=P),
                    in1=dc3, op=mybir.AluOpType.is_equal)
                nc.vector.tensor_tensor(
                    out=swa3, in0=swa3, in1=w3, op=mybir.AluOpType.mult)
                for t in range(tT):
                    nc.tensor.matmul(
                        ps[:], lhsT=swa[:, t * P:(t + 1) * P],
                        rhs=G[:, t * rowlen:t * rowlen + n_mm],
                        start=(t == 0), stop=(t == tT - 1))
                dn = s_pool.tile([P, 1], F32, tag="dn")
                nc.vector.tensor_scalar_add(dn[:], ps[:, C_out:C_out + 1],
                                            1e-30)
                rc = s_pool.tile([P, 1], F32, tag="rc")
                nc.vector.reciprocal(rc[:], dn[:])
                ar = a_pool.tile([P, C_out], F32, tag="ar")
                nc.scalar.activation(ar[:], ps[:, 0:C_out],
                                     mybir.ActivationFunctionType.Copy,
                                     scale=rc[:])
                nc.vector.tensor_tensor(out=ar[:], in0=ar[:], in1=bbc[:],
                                        op=mybir.AluOpType.add)
                if layer < 3:
                    ar2 = a_pool.tile([P, C_out], F32, tag="ar2")
                    nc.vector.scalar_tensor_tensor(
                        out=ar2[:], in0=ar[:], scalar=NEG_ACT, in1=ar[:],
                        op0=mybir.AluOpType.mult, op1=mybir.AluOpType.max)
                    for k in range(2):
                        tp = ps_tp.tile([P, P], F32, tag="tp")
                        nc.tensor.transpose(tp[:], ar2[:, k * P:(k + 1) * P],
                                            ident[:])
                        nc.scalar.copy(
                            out=aT[k].ap()[:, b * P:(b + 1) * P], in_=tp[:])
                else:
                    nc.sync.dma_start(out=out_d[:, :], in_=ar[:])

        # ====================== the network ======================
        for _rep in range(repeat):
            bbc1 = bias_bcast(b1, HID, "b1")
            dense_phase(1, [w1_sb], h1shard, stab[0], ROW, tdt,
                        one_col=HID, sd_col=HID + 2)
            if not SKIP_COLL:
                nc.gpsimd.collective_compute(
                    "AllGather", mybir.AluOpType.bypass, replica_groups=rg,
                    ins=[h1shard.ap()], outs=[h1tab.ap()])
            agg_phase(1, h1tab, stab[0], NB, TA, TB, ROW, tdt,
                      epack, HID, bbc1)

            bbc2 = bias_bcast(b2, HID, "b2")
            dense_phase(2, w2_sb, h2shard, stab[1], ROW, tdt,
                        one_col=HID, sd_col=HID + 2)
            if not SKIP_COLL:
                nc.gpsimd.collective_compute(
                    "AllGather", mybir.AluOpType.bypass, replica_groups=rg,
                    ins=[h2shard.ap()], outs=[h2tab.ap()])
            agg_phase(2, h2tab, stab[1], NB, TA, TB, ROW, tdt,
                      epack, HID, bbc2)

            bbc3 = bias_bcast(b3, OUT, "b3")
            dense_phase(3, w3_sb, h3shard, stab[2], ROW3, F32,
                        one_col=OUT, sd_col=OUT + 2)
            if not SKIP_COLL:
                nc.gpsimd.collective_compute(
                    "AllGather", mybir.AluOpType.bypass, replica_groups=rg,
                    ins=[h3shard.ap()], outs=[h3tab.ap()])
            agg_phase(3, h3tab, stab[2], 1, T3A, T3B, ROW3, F32,
                      epack3, OUT, bbc3)

    nc.compile()
    return nc


# ----------------------------------------------------------------------------
# host wrapper
# ----------------------------------------------------------------------------

def make_in_maps(inputs, pre, cfg: Cfg):
    R = cfg.rows
    N = cfg.n_nodes
    x = np.asarray(inputs["x"], np.float32)

    def wext(W, a_s, a_d):
        W = np.asarray(W, np.float32)
        a_s = np.asarray(a_s, np.float32)
        a_d = np.asarray(a_d, np.float32)
        z = np.zeros((W.shape[0], 1), np.float32)
        return np.concatenate(
            [W, z, (W @ a_s)[:, None], (W @ a_d)[:, None]], axis=1
        ).astype(np.float32)

    W1e = wext(inputs["W1"], inputs["a_src1"], inputs["a_dst1"])
    W2e = wext(inputs["W2"], inputs["a_src2"], inputs["a_dst2"])
    W3e = wext(inputs["W3"], inputs["a_src3"], inputs["a_dst3"])
    b1 = np.asarray(inputs["b1"], np.float32).reshape(1, -1)
    b2 = np.asarray(inputs["b2"], np.float32).reshape(1, -1)
    b3 = np.asarray(inputs["b3"], np.float32).reshape(1, -1)
    in_maps = []
    for c in range(NCORES):
        lo, hi = c * R, min((c + 1) * R, N)
        xs = np.zeros((P, R), np.float32)
        xs[:, 0:hi - lo] = x[lo:hi].T
        in_maps.append({
            "xT": xs, "W1e": W1e, "W2e": W2e, "W3e": W3e,
            "b1": b1, "b2": b2, "b3": b3,
            "epack": pre["epack"][c], "epack3": pre["epack3"][c],
        })
    return in_maps


_CACHE = {}


def get_program(cfg: Cfg, TA, TB, T3A, T3B, repeat=1):
    key = (cfg.n_nodes, cfg.use_bf16_tab, TA, TB, T3A, T3B, repeat)
    if key not in _CACHE:
        _CACHE[key] = build_program(cfg, TA, TB, T3A, T3B, repeat)
    return _CACHE[key]


def run(inputs, cfg: Cfg, trace=False):
    pre = preprocess(inputs["edge_src"], inputs["edge_dst"], cfg)
    in_maps = make_in_maps(inputs, pre, cfg)
    nc = get_program(cfg, pre["TA"], pre["TB"], pre["T3A"], pre["T3B"])
    res = run_bass_kernel_spmd(nc, in_maps, list(range(NCORES)), trace=trace)
    outs = []
    for c in range(NCORES):
        outs.append(res.results[c]["out"][0:pre["zcounts"][c], :])
    return np.concatenate(outs, axis=0).astype(np.float32), res


def kernel(**inputs):
    cfg = Cfg(n_nodes=inputs["x"].shape[0],
              in_ch=inputs["x"].shape[1],
              hid=inputs["W1"].shape[1],
              out_ch=inputs["W3"].shape[1],
              use_bf16_tab=os.environ.get("GAT_BF16", "0") == "1")
    out, _ = run(inputs, cfg)
    return out



# revision 13
# speedup vs baseline: 1.4240x; 1.4240x over previous
"""3-layer GAT (GATNet) on 8 Trainium2 NeuronCores via Bass/Tile. (v2)

Sharding: nodes (and their incident edges, grouped by destination) are
partitioned across the 8 cores; weights are replicated. Per layer:
  dense:  H_ext = A @ W_ext for this core's node shard.  W_ext carries
          extra columns so the same matmul also produces the attention
          scores s_src/s_dst per node.  Rows are written to a gather
          table as [h | 1.0 | s_src | pad] (row bytes % 256 == 0).
          s_dst stays in SBUF (sdall column per block).
  AllGather of the gather table across the 8 cores.
  agg:    per chunk of dst blocks, dma_gather the source-node rows of
          the chunk's edges (two gathers: src-halves A/B since
          dma_gather indices are int16), build the selection mask
          Se[j,d] = (dst_local[j]==d) with one DVE pass against an iota
          row, recover per-edge s_dst with Se*bcast(s_dst) + reduce
          (instead of a per-edge 256B dma_gather - the per-edge
          descriptors dominated HW time), then
          w = exp(leaky_relu(s_src+s_dst)), swa = Se*w, and accumulate
          PSUM[d, :] += swa^T @ G on the PE.  The constant-1 column in
          each row makes the same matmuls produce the softmax
          denominators.  Epilogue: divide, add bias, leaky_relu, and
          PE-transpose into the next layer's lhsT chunks (kept in SBUF).
Layer 3 aggregates only for the output nodes (first node of each graph),
using the old per-edge s_dst gather (single block, negligible).
"""

import os
import sys

import numpy as np

sys.path.insert(0, "/opt/trn_rl_repo")

from contextlib import ExitStack  # noqa: E402

from concourse import bacc, bass, mybir, tile  # noqa: E402
from concourse.bass_utils import run_bass_kernel_spmd  # noqa: E402
from concourse.masks import make_identity  # noqa: E402

F32 = mybir.dt.float32
BF16 = mybir.dt.bfloat16
I16 = mybir.dt.int16
I32 = mybir.dt.int32

P = 128
NCORES = 8
NEG_ATT = 0.2
NEG_ACT = 0.01

CH = int(os.environ.get("GAT_CHUNK", "1"))  # dst blocks per gather call
QSPREAD = os.environ.get("GAT_QSPREAD", "0") == "1"  # SWDGE queue spread
NEGPAD = os.environ.get("GAT_NEGPAD", "0") == "1"  # -1 pad idxs skip descs
SORTSRC = os.environ.get("GAT_SORT", "0") == "1"   # sort edges by src
ACTSWA = os.environ.get("GAT_ACTSWA", "1") == "1"  # swa multiply on ACT


def _ceil(a, b):
    return -(-a // b)


class Cfg:
    def __init__(self, n_nodes=50000, in_ch=128, hid=256, out_ch=16,
                 graph=50):
        assert in_ch == 128
        self.n_nodes = n_nodes
        self.in_ch = in_ch
        self.hid = hid
        self.out_ch = out_ch
        self.graph = graph
        self.nb = _ceil(_ceil(n_nodes, NCORES), P)   # dst blocks per core
        self.rows = self.nb * P                      # padded rows per core
        self.ntot = self.rows * NCORES
        self.row12 = 320              # f32: 1280 B (row bytes % 256 == 0)
        self.row3 = 64                # f32: 256 B
        self.split = min(32768, ((self.ntot // 2) // P) * P)
        assert self.split <= 32768 and self.ntot - self.split <= 32768


# ----------------------------------------------------------------------------
# host-side edge preprocessing
# ----------------------------------------------------------------------------

def preprocess(edge_src, edge_dst, cfg: Cfg):
    """Build the per-core edge-stream arrays.

    L1/L2 stream: edges grouped by (core, dst-block, region) where region
    A/B = src </>= SPLIT.  Slot s of a region maps to gathered-tile
    position (partition s%128, tile s//128).  Padding slots use gather
    index 0 and dst_local 999 (never matches).  epack per block:
    [gidx (8T cols) | dstcol-as-i16 (2T cols)].

    L3 stream: only edges into the zero-mask nodes; keeps the s_dst
    per-edge gather (dloc), epack3 = [gidx|dloc|dstcol] (18*T3 cols)."""
    src = np.asarray(edge_src).astype(np.int64)
    dst = np.asarray(edge_dst).astype(np.int64)
    R, NB = cfg.rows, cfg.nb
    N = cfg.n_nodes

    core = dst // R
    blk = (dst - core * R) // P
    SPLIT = cfg.split
    reg = (src >= SPLIT).astype(np.int64)

    # ---- layers 1/2 stream: group edges by (core, block, region) ----
    key = (core * NB + blk) * 2 + reg
    if SORTSRC:
        order = np.lexsort((src, key))
    else:
        order = np.argsort(key, kind="stable")
    ks, ss, ds = key[order], src[order], dst[order]
    starts = np.searchsorted(ks, np.arange(2 * NCORES * NB))
    pos = np.arange(len(ks)) - starts[ks]

    nA = np.zeros((NCORES, NB), np.int64)
    nB = np.zeros((NCORES, NB), np.int64)
    np.add.at(nA, (core, blk), 1 - reg)
    np.add.at(nB, (core, blk), reg)
    TA = max(1, _ceil(int(nA.max()), P))
    TB = max(1, _ceil(int(nB.max()), P))
    T = TA + TB

    c_s = ks // (2 * NB)
    b_s = (ks // 2) % NB
    r_s = ks % 2
    slot = np.where(r_s == 0, pos, TA * P + pos)
    p_s = slot % P
    t_s = slot // P

    fill = -1 if NEGPAD else 0
    gidx16 = np.full((NCORES, NB, 16, T * 8), fill, np.int16)
    dstcol = np.full((NCORES, NB, P, T), 999.0, np.float32)

    wcol = np.where(r_s == 0, pos // 16, TA * 8 + pos // 16)
    val = np.where(r_s == 0, ss, ss - SPLIT).astype(np.int16)
    gidx16[c_s, b_s, pos % 16, wcol] = val
    dstcol[c_s, b_s, p_s, t_s] = ((ds - c_s * R) % P).astype(np.float32)

    gidx = np.tile(gidx16, (1, 1, 8, 1))  # replicate across the 8 Q7 groups
    epack = np.concatenate([gidx, dstcol.view(np.int16)], axis=-1)

    # ---- layer-3 stream: only edges into the zero-mask (output) nodes ----
    nodes = np.arange(N)
    zmask_node = (nodes % cfg.graph) == 0
    zcounts = np.bincount(nodes[zmask_node] // R, minlength=NCORES)
    zslot_of = np.full(N, -1, np.int64)
    for c in range(NCORES):
        zn = nodes[zmask_node & (nodes // R == c)]
        zslot_of[zn] = np.arange(len(zn))

    sel = zmask_node[dst]
    s3, d3 = src[sel], dst[sel]
    c3 = d3 // R
    r3 = (s3 >= SPLIT).astype(np.int64)
    key3 = c3 * 2 + r3
    o3 = np.argsort(key3, kind="stable")
    k3, s3, d3 = key3[o3], s3[o3], d3[o3]
    starts3 = np.searchsorted(k3, np.arange(2 * NCORES))
    pos3 = np.arange(len(k3)) - starts3[k3]
    n3A = np.zeros(NCORES, np.int64)
    n3B = np.zeros(NCORES, np.int64)
    np.add.at(n3A, c3, 1 - r3)
    np.add.at(n3B, c3, r3)
    T3A = max(1, _ceil(int(n3A.max()), P))
    T3B = max(1, _ceil(int(n3B.max()), P))
    T3 = T3A + T3B

    cc3 = k3 // 2
    rr3 = k3 % 2
    slot3 = np.where(rr3 == 0, pos3, T3A * P + pos3)
    p3 = slot3 % P
    t3 = slot3 // P
    gidx316 = np.full((NCORES, 16, T3 * 8), fill, np.int16)
    dloc316 = np.full((NCORES, 16, T3 * 8), fill, np.int16)
    dstcol3 = np.full((NCORES, P, T3), 999.0, np.float32)
    wcol3 = np.where(rr3 == 0, pos3 // 16, T3A * 8 + pos3 // 16)
    val3 = np.where(rr3 == 0, s3, s3 - SPLIT).astype(np.int16)
    gidx316[cc3, pos3 % 16, wcol3] = val3
    rl3 = d3 - cc3 * R
    dloc316[cc3, slot3 % 16, slot3 // 16] = rl3.astype(np.int16)
    dstcol3[cc3, p3, t3] = zslot_of[d3].astype(np.float32)
    gidx3 = np.tile(gidx316, (1, 8, 1))
    dloc3 = np.tile(dloc316, (1, 8, 1))
    epack3 = np.concatenate(
        [gidx3, dloc3, dstcol3.view(np.int16)], axis=-1)

    return dict(TA=TA, TB=TB, T3A=T3A, T3B=T3B,
                epack=epack, epack3=epack3,
                zcounts=zcounts)


# ----------------------------------------------------------------------------
# program builder
# ----------------------------------------------------------------------------

def build_program(cfg: Cfg, TA, TB, T3A, T3B, repeat=1, queue_map=None):
    NB, R = cfg.nb, cfg.rows
    T, T3 = TA + TB, T3A + T3B
    ROW, ROW3 = cfg.row12, cfg.row3
    HID, OUT = cfg.hid, cfg.out_ch
    NTOT = cfg.ntot
    SPLIT = cfg.split
    SROW = 64

    nc = bacc.Bacc("TRN2", target_bir_lowering=False, debug=False,
                   num_devices=NCORES,
                   num_swdge_queues=4 if QSPREAD else 1)

    # ---- I/O ----
    xT = nc.dram_tensor("xT", [P, R], F32, kind="ExternalInput")
    W1e = nc.dram_tensor("W1e", [P, HID + 3], F32, kind="ExternalInput")
    W2e = nc.dram_tensor("W2e", [HID, HID + 3], F32, kind="ExternalInput")
    W3e = nc.dram_tensor("W3e", [HID, OUT + 3], F32, kind="ExternalInput")
    b1 = nc.dram_tensor("b1", [1, HID], F32, kind="ExternalInput")
    b2 = nc.dram_tensor("b2", [1, HID], F32, kind="ExternalInput")
    b3 = nc.dram_tensor("b3", [1, OUT], F32, kind="ExternalInput")
    epack = nc.dram_tensor("epack", [NB, P, T * 10], I16,
                           kind="ExternalInput")
    epack3 = nc.dram_tensor("epack3", [P, T3 * 18], I16,
                            kind="ExternalInput")
    out_d = nc.dram_tensor("out", [P, OUT], F32, kind="ExternalOutput")

    # ---- internal DRAM ----
    h1shard = nc.dram_tensor("h1shard", [R, ROW], F32)
    h1tab = nc.dram_tensor("h1tab", [NTOT, ROW], F32, addr_space="Shared")
    h2shard = nc.dram_tensor("h2shard", [R, ROW], F32)
    h2tab = nc.dram_tensor("h2tab", [NTOT, ROW], F32, addr_space="Shared")
    h3shard = nc.dram_tensor("h3shard", [R, ROW3], F32)
    h3tab = nc.dram_tensor("h3tab", [NTOT, ROW3], F32, addr_space="Shared")
    stab3 = nc.dram_tensor("stab3", [R, SROW], F32)

    rg = [list(range(NCORES))]

    # persistent next-layer lhsT chunks (A^T), reused across layer pairs;
    # aT[0] doubles as the layer-1 input x^T
    aT = [nc.alloc_sbuf_tensor("aT0", [P, R], F32),
          nc.alloc_sbuf_tensor("aT1", [P, R], F32)]

    with tile.TileContext(nc) as tc, ExitStack() as ctx:
        cpool = ctx.enter_context(tc.tile_pool(name="const", bufs=1))
        wpool = ctx.enter_context(tc.tile_pool(name="weights", bufs=1))
        row_pool = ctx.enter_context(tc.tile_pool(name="rows", bufs=3))
        idx_pool = ctx.enter_context(tc.tile_pool(name="idx", bufs=4))
        g_pool = ctx.enter_context(tc.tile_pool(name="gather", bufs=2))
        s_pool = ctx.enter_context(tc.tile_pool(name="scal", bufs=4))
        se_pool = ctx.enter_context(tc.tile_pool(name="sew", bufs=2))
        td_pool = ctx.enter_context(tc.tile_pool(name="tmpd", bufs=1))
        sd_pool = ctx.enter_context(tc.tile_pool(name="sdbc", bufs=2))
        a_pool = ctx.enter_context(tc.tile_pool(name="arow", bufs=3))
        ps_dense = ctx.enter_context(
            tc.tile_pool(name="psd", bufs=2, space="PSUM"))
        ps_agg = ctx.enter_context(
            tc.tile_pool(name="psa", bufs=2, space="PSUM"))
        ps_tp = ctx.enter_context(
            tc.tile_pool(name="pst", bufs=1, space="PSUM"))
        ps_bc = ctx.enter_context(
            tc.tile_pool(name="psb", bufs=1, space="PSUM"))

        # constants
        ident = cpool.tile([P, P], F32, tag="ident")
        make_identity(nc, ident[:])
        ones128 = cpool.tile([P, P], F32, tag="ones128")
        nc.vector.memset(ones128[:], 1.0)
        TMAX = max(CH * T, T3)
        iota_i = cpool.tile([P, TMAX * P], I32, tag="iotai")
        nc.gpsimd.iota(iota_i[:], pattern=[[0, TMAX], [1, P]], base=0,
                       channel_multiplier=0)
        iota_f = cpool.tile([P, TMAX * P], F32, tag="iotaf")
        nc.vector.tensor_copy(out=iota_f[:], in_=iota_i[:])
        ones1 = cpool.tile([1, P], F32, tag="ones1")
        nc.vector.memset(ones1[:], 1.0)

        # preload weights
        w1_sb = wpool.tile([P, HID + 3], F32, tag="w1")
        nc.sync.dma_start(out=w1_sb[:], in_=W1e[:, :])
        w2_sb = [wpool.tile([P, HID + 3], F32, tag=f"w2_{k}",
                            name=f"w2sb{k}") for k in range(2)]
        for k in range(2):
            nc.sync.dma_start(out=w2_sb[k][:], in_=W2e[k * P:(k + 1) * P, :])
        w3_sb = [wpool.tile([P, OUT + 3], F32, tag=f"w3_{k}",
                            name=f"w3sb{k}") for k in range(2)]
        for k in range(2):
            nc.sync.dma_start(out=w3_sb[k][:], in_=W3e[k * P:(k + 1) * P, :])

        # SWDGE queue assignment: tile_sem_assignment round-robins Pool DMAs
        # over 8 DMASW lanes in SCHEDULED order, and each lane's semaphore
        # is locked to one queue.  queue_map (from a first build pass) maps
        # issue-index -> queue so that queue == lane % 4.
        qctr = [0]

        def qn():
            i = qctr[0]
            qctr[0] += 1
            if queue_map is None:
                return 0
            return queue_map[i]

        def bias_bcast(bd, C, tag):
            brow = cpool.tile([1, C], F32, tag=f"brow_{tag}")
            nc.sync.dma_start(out=brow[:], in_=bd[:, :])
            bps = ps_bc.tile([P, C], F32, tag="bps")
            nc.tensor.matmul(bps[:], lhsT=ones1[:], rhs=brow[:],
                             start=True, stop=True)
            bbc = cpool.tile([P, C], F32, tag=f"bbc_{tag}")
            nc.vector.tensor_copy(out=bbc[:], in_=bps[:])
            return bbc

        # ------------------------------------------------------------------
        def dense_phase(layer, w_tiles, shard, sd_col, sdall, sdacc_dst,
                        rowlen, one_col):
            """H_ext = A @ W_ext for this core's rows; writes the gather
            table rows [h | 1 | s_src | pad]; s_dst goes to the sdall SBUF
            column (L1/2) or a DRAM stab (L3)."""
            for it in range(NB):
                if layer == 1:
                    lts = [aT[0].ap()[:, it * P:(it + 1) * P]]
                else:
                    lts = [aT[k].ap()[:, it * P:(it + 1) * P]
                           for k in range(2)]
                ps = ps_dense.tile([P, one_col + 3], F32, tag="dps")
                for k, lt_ap in enumerate(lts):
                    nc.tensor.matmul(ps[:], lhsT=lt_ap, rhs=w_tiles[k][:],
                                     start=(k == 0), stop=(k == len(lts) - 1))
                row = row_pool.tile([P, rowlen], F32, tag=f"row{rowlen}")
                # psum col one_col is 0 (zero column in W_ext);
                # col one_col+1 is s_src
                nc.vector.tensor_copy(out=row[:, 0:one_col + 2],
                                      in_=ps[:, 0:one_col + 2])
                nc.vector.memset(row[:, one_col:one_col + 1], 1.0)
                nc.vector.memset(row[:, one_col + 2:rowlen], 0.0)
                if sdall is not None:
                    nc.vector.tensor_copy(
                        out=sdall[:, it:it + 1],
                        in_=ps[:, sd_col:sd_col + 1])
                else:
                    srow = row_pool.tile([P, SROW], F32, tag="srow")
                    nc.vector.memset(srow[:, 1:SROW], 0.0)
                    nc.vector.tensor_copy(out=srow[:, 0:1],
                                          in_=ps[:, sd_col:sd_col + 1])
                    nc.sync.dma_start(
                        out=sdacc_dst[it * P:(it + 1) * P, :], in_=srow[:])
                nc.sync.dma_start(out=shard[it * P:(it + 1) * P, :],
                                  in_=row[:])

        # ------------------------------------------------------------------
        def agg12(layer, tab, sdall, epack_t, bbc):
            """L1/L2 aggregation: chunks of CH blocks, s_dst on-chip."""
            n_mm = HID + 1
            tabA = tab[0:SPLIT, :]
            tabB = tab[SPLIT:NTOT, :]
            nch = _ceil(NB, CH)
            for c in range(nch):
                b0 = c * CH
                nb_c = min(CH, NB - b0)  # blocks in this chunk
                tc_a, tc_b = nb_c * TA, nb_c * TB
                ct = tc_a + tc_b
                ep = idx_pool.tile([P, CH * T * 10], I16, tag="ep")
                for j in range(nb_c):
                    nc.sync.dma_start(
                        out=ep[:, j * T * 10:(j + 1) * T * 10],
                        in_=epack_t[b0 + j, :, :])
                # chunk-level idx layout (host side groups per block):
                # [gidxA(b) | gidxB(b) | dc(b)] per block, consecutively.
                G = g_pool.tile([P, CH * T * ROW], F32, tag="G")
                if NEGPAD and layer == 1 and c < 2:
                    # -1 pad idxs skip their descriptors, leaving pad slots
                    # unwritten; scrub the two G buffers once so skipped
                    # slots never hold NaN bit patterns (0*NaN poisons PSUM)
                    nc.vector.memset(G[:], 0.0)
                Se = se_pool.tile([P, CH * T * P], F32, tag="Se")
                tmpD = td_pool.tile([P, CH * T * P], F32, tag="tmpD")
                sdbc = sd_pool.tile([P, nb_c * P], F32, tag="sdbc")
                ssd = s_pool.tile([P, CH * T], F32, tag="ssd")
                z = s_pool.tile([P, CH * T], F32, tag="z")
                w = s_pool.tile([P, CH * T], F32, tag="w")
                for j in range(nb_c):
                    b = b0 + j
                    ebase = j * T * 10
                    G3d = G[:, j * T * ROW:(j + 1) * T * ROW].rearrange(
                        "p (t c) -> p t c", c=ROW)
                    nc.gpsimd.dma_gather(
                        out_ap=G3d[:, 0:TA, :], in_ap=tabA,
                        idxs_ap=ep[:, ebase:ebase + TA * 8],
                        num_idxs=TA * P, num_idxs_reg=TA * P,
                        elem_size=ROW, elem_step=ROW, single_packet=False,
                        queue_num=qn())
                    nc.gpsimd.dma_gather(
                        out_ap=G3d[:, TA:T, :], in_ap=tabB,
                        idxs_ap=ep[:, ebase + TA * 8:ebase + T * 8],
                        num_idxs=TB * P, num_idxs_reg=TB * P,
                        elem_size=ROW, elem_step=ROW, single_packet=False,
                        queue_num=qn())
                    # s_dst broadcast for this block:
                    # diag = ident * sdall[:,b]; bcast = ones^T @ diag
                    diag = sd_pool.tile([P, P], F32, tag="diag")
                    nc.vector.tensor_scalar_mul(
                        diag[:], ident[:], sdall[:, b:b + 1])
                    bps = ps_bc.tile([P, P], F32, tag="sdps")
                    nc.tensor.matmul(bps[:], lhsT=ones128[:], rhs=diag[:],
                                     start=True, stop=True)
                    nc.scalar.copy(out=sdbc[:, j * P:(j + 1) * P],
                                   in_=bps[:])
                    # selection mask for this block's slots
                    dcj = ep[:, ebase + T * 8:ebase + T * 10].bitcast(F32)
                    se3 = Se[:, j * T * P:(j + 1) * T * P].rearrange(
                        "p (t d) -> p t d", d=P)
                    dc3 = dcj.unsqueeze(-1).to_broadcast([P, T, P])
                    nc.vector.tensor_tensor(
                        out=se3, in0=iota_f[:, 0:T * P].rearrange(
                            "p (t d) -> p t d", d=P),
                        in1=dc3, op=mybir.AluOpType.is_equal)
                    # per-edge s_dst = reduce_d(Se * bcast(s_dst))
                    td3 = tmpD[:, j * T * P:(j + 1) * T * P].rearrange(
                        "p (t d) -> p t d", d=P)
                    sd3 = sdbc[:, j * P:(j + 1) * P].unsqueeze(
                        1).to_broadcast([P, T, P])
                    nc.vector.tensor_tensor(out=td3, in0=se3, in1=sd3,
                                            op=mybir.AluOpType.mult)
                    nc.vector.tensor_reduce(
                        out=ssd[:, j * T:(j + 1) * T], in_=td3,
                        axis=mybir.AxisListType.X, op=mybir.AluOpType.add)
                # w = exp(leaky(s_src + s_dst)) for the whole chunk
                ssrc = G[:, HID + 1::ROW]
                nc.vector.tensor_tensor(out=z[:, 0:nb_c * T],
                                        in0=ssrc[:, 0:nb_c * T],
                                        in1=ssd[:, 0:nb_c * T],
                                        op=mybir.AluOpType.add)
                e = s_pool.tile([P, CH * T], F32, tag="e")
                nc.vector.scalar_tensor_tensor(
                    out=e[:, 0:nb_c * T], in0=z[:, 0:nb_c * T],
                    scalar=NEG_ATT, in1=z[:, 0:nb_c * T],
                    op0=mybir.AluOpType.mult, op1=mybir.AluOpType.max)
                nc.scalar.activation(w[:, 0:nb_c * T], e[:, 0:nb_c * T],
                                     mybir.ActivationFunctionType.Exp)
                # swa = Se * w  (in place over Se)
                if ACTSWA:
                    for tt in range(nb_c * T):
                        nc.scalar.activation(
                            Se[:, tt * P:(tt + 1) * P],
                            Se[:, tt * P:(tt + 1) * P],
                            mybir.ActivationFunctionType.Copy,
                            scale=w[:, tt:tt + 1])
                else:
                    se3c = Se[:, 0:nb_c * T * P].rearrange(
                        "p (t d) -> p t d", d=P)
                    w3 = w[:, 0:nb_c * T].unsqueeze(-1).to_broadcast(
                        [P, nb_c * T, P])
                    nc.vector.tensor_tensor(out=se3c, in0=se3c, in1=w3,
                                            op=mybir.AluOpType.mult)
                # matmuls + epilogue per block
                for j in range(nb_c):
                    b = b0 + j
                    ps = ps_agg.tile([P, n_mm], F32, tag=f"aps{j}",
                                     name=f"aps{j}")
                    for t in range(T):
                        tt = j * T + t
                        nc.tensor.matmul(
                            ps[:], lhsT=Se[:, tt * P:(tt + 1) * P],
                            rhs=G[:, tt * ROW:tt * ROW + n_mm],
                            start=(t == 0), stop=(t == T - 1))
                    dn = s_pool.tile([P, 1], F32, tag="dn")
                    nc.vector.tensor_scalar_add(dn[:], ps[:, HID:HID + 1],
                                                1e-30)
                    rc = s_pool.tile([P, 1], F32, tag="rc")
                    nc.vector.reciprocal(rc[:], dn[:])
                    ar = a_pool.tile([P, HID], F32, tag="ar")
                    nc.scalar.activation(ar[:], ps[:, 0:HID],
                                         mybir.ActivationFunctionType.Copy,
                                         scale=rc[:])
                    nc.vector.tensor_tensor(out=ar[:], in0=ar[:], in1=bbc[:],
                                            op=mybir.AluOpType.add)
                    ar2 = a_pool.tile([P, HID], F32, tag="ar2")
                    nc.vector.scalar_tensor_tensor(
                        out=ar2[:], in0=ar[:], scalar=NEG_ACT, in1=ar[:],
                        op0=mybir.AluOpType.mult, op1=mybir.AluOpType.max)
                    for k in range(2):
                        tp = ps_tp.tile([P, P], F32, tag="tp")
                        nc.tensor.transpose(tp[:], ar2[:, k * P:(k + 1) * P],
                                            ident[:])
                        nc.scalar.copy(
                            out=aT[k].ap()[:, b * P:(b + 1) * P], in_=tp[:])

        # ------------------------------------------------------------------
        def agg3(tab, sd_t, bbc):
            """L3: single dst block for the output nodes; per-edge s_dst
            via dma_gather (negligible size)."""
            tT = T3A + T3B
            n_mm = OUT + 1
            tabA = tab[0:SPLIT, :]
            tabB = tab[SPLIT:NTOT, :]
            ep = idx_pool.tile([P, tT * 18], I16, tag="ep3")
            nc.sync.dma_start(out=ep[:], in_=epack3[:, :])
            gi = ep[:, 0:tT * 8]
            dl = ep[:, tT * 8:tT * 16]
            dc = ep[:, tT * 16:tT * 18].bitcast(F32)
            G = g_pool.tile([P, tT * ROW3], F32, tag="G3")
            if NEGPAD:
                nc.vector.memset(G[:], 0.0)
            G3d = G[:].rearrange("p (t c) -> p t c", c=ROW3)
            nc.gpsimd.dma_gather(
                out_ap=G3d[:, 0:T3A, :], in_ap=tabA,
                idxs_ap=gi[:, 0:T3A * 8], num_idxs=T3A * P,
                num_idxs_reg=T3A * P, elem_size=ROW3, elem_step=ROW3,
                single_packet=False, queue_num=qn())
            nc.gpsimd.dma_gather(
                out_ap=G3d[:, T3A:tT, :], in_ap=tabB,
                idxs_ap=gi[:, T3A * 8:tT * 8], num_idxs=T3B * P,
                num_idxs_reg=T3B * P, elem_size=ROW3, elem_step=ROW3,
                single_packet=False, queue_num=qn())
            Gs = g_pool.tile([P, tT * SROW], F32, tag="Gs3")
            if NEGPAD:
                nc.vector.memset(Gs[:], 0.0)
            Gs3d = Gs[:].rearrange("p (t c) -> p t c", c=SROW)
            nc.gpsimd.dma_gather(
                out_ap=Gs3d[:, :, :], in_ap=sd_t[:, :],
                idxs_ap=dl, num_idxs=tT * P,
                num_idxs_reg=tT * P, elem_size=SROW, elem_step=SROW,
                single_packet=False, queue_num=qn())
            sdp = Gs[:, 0::SROW]
            ssrc = G[:, OUT + 1::ROW3]
            z = s_pool.tile([P, tT], F32, tag="z3")
            nc.vector.tensor_tensor(out=z[:], in0=ssrc, in1=sdp,
                                    op=mybir.AluOpType.add)
            e = s_pool.tile([P, tT], F32, tag="e3")
            nc.vector.scalar_tensor_tensor(
                out=e[:], in0=z[:], scalar=NEG_ATT, in1=z[:],
                op0=mybir.AluOpType.mult, op1=mybir.AluOpType.max)
            w = s_pool.tile([P, tT], F32, tag="w3")
            nc.scalar.activation(w[:], e[:],
                                 mybir.ActivationFunctionType.Exp)
            ps = ps_agg.tile([P, n_mm], F32, tag="aps0")
            swa = se_pool.tile([P, tT * P], F32, tag="swa3")
            dc3 = dc.unsqueeze(-1).to_broadcast([P, tT, P])
            w3 = w[:].unsqueeze(-1).to_broadcast([P, tT, P])
            swa3 = swa[:].rearrange("p (t d) -> p t d", d=P)
            nc.vector.tensor_tensor(
                out=swa3, in0=iota_f[:, 0:tT * P].rearrange(
                    "p (t d) -> p t d", d=P),
                in1=dc3, op=mybir.AluOpType.is_equal)
            nc.vector.tensor_tensor(
                out=swa3, in0=swa3, in1=w3, op=mybir.AluOpType.mult)
            for t in range(tT):
                nc.tensor.matmul(
                    ps[:], lhsT=swa[:, t * P:(t + 1) * P],
                    rhs=G[:, t * ROW3:t * ROW3 + n_mm],
                    start=(t == 0), stop=(t == tT - 1))
            dn = s_pool.tile([P, 1], F32, tag="dn3")
            nc.vector.tensor_scalar_add(dn[:], ps[:, OUT:OUT + 1], 1e-30)
            rc = s_pool.tile([P, 1], F32, tag="rc3")
            nc.vector.reciprocal(rc[:], dn[:])
            ar = a_pool.tile([P, OUT], F32, tag="ar3")
            nc.scalar.activation(ar[:], ps[:, 0:OUT],
                                 mybir.ActivationFunctionType.Copy,
                                 scale=rc[:])
            nc.vector.tensor_tensor(out=ar[:], in0=ar[:], in1=bbc[:],
                                    op=mybir.AluOpType.add)
            nc.sync.dma_start(out=out_d[:, :], in_=ar[:])

        # ====================== the network ======================
        for _rep in range(repeat):
            # layer-1 lhsT = x^T (reloaded per rep: agg overwrites aT)
            nc.sync.dma_start(out=aT[0].ap()[:, :], in_=xT[:, :])
            sdall1 = cpool.tile([P, NB], F32, tag="sdall1", name="sdall1")
            bbc1 = bias_bcast(b1, HID, "b1")
            dense_phase(1, [w1_sb], h1shard, HID + 2, sdall1, None,
                        ROW, HID)
            nc.gpsimd.collective_compute(
                "AllGather", mybir.AluOpType.bypass, replica_groups=rg,
                ins=[h1shard.ap()], outs=[h1tab.ap()])
            agg12(1, h1tab, sdall1, epack, bbc1)

            sdall2 = cpool.tile([P, NB], F32, tag="sdall2", name="sdall2")
            bbc2 = bias_bcast(b2, HID, "b2")
            dense_phase(2, w2_sb, h2shard, HID + 2, sdall2, None,
                        ROW, HID)
            nc.gpsimd.collective_compute(
                "AllGather", mybir.AluOpType.bypass, replica_groups=rg,
                ins=[h2shard.ap()], outs=[h2tab.ap()])
            agg12(2, h2tab, sdall2, epack, bbc2)

            bbc3 = bias_bcast(b3, OUT, "b3")
            dense_phase(3, w3_sb, h3shard, OUT + 2, None, stab3,
                        ROW3, OUT)
            nc.gpsimd.collective_compute(
                "AllGather", mybir.AluOpType.bypass, replica_groups=rg,
                ins=[h3shard.ap()], outs=[h3tab.ap()])
            agg3(h3tab, stab3, bbc3)

    nc.compile()
    return nc


# ----------------------------------------------------------------------------
# host wrapper
# ----------------------------------------------------------------------------

def make_in_maps(inputs, pre, cfg: Cfg):
    R = cfg.rows
    N = cfg.n_nodes
    x = np.asarray(inputs["x"], np.float32)

    def wext(W, a_s, a_d):
        W = np.asarray(W, np.float32)
        a_s = np.asarray(a_s, np.float32)
        a_d = np.asarray(a_d, np.float32)
        z = np.zeros((W.shape[0], 1), np.float32)
        return np.concatenate(
            [W, z, (W @ a_s)[:, None], (W @ a_d)[:, None]], axis=1
        ).astype(np.float32)

    W1e = wext(inputs["W1"], inputs["a_src1"], inputs["a_dst1"])
    W2e = wext(inputs["W2"], inputs["a_src2"], inputs["a_dst2"])
    W3e = wext(inputs["W3"], inputs["a_src3"], inputs["a_dst3"])
    b1 = np.asarray(inputs["b1"], np.float32).reshape(1, -1)
    b2 = np.asarray(inputs["b2"], np.float32).reshape(1, -1)
    b3 = np.asarray(inputs["b3"], np.float32).reshape(1, -1)
    in_maps = []
    for c in range(NCORES):
        lo, hi = c * R, min((c + 1) * R, N)
        xs = np.zeros((P, R), np.float32)
        xs[:, 0:hi - lo] = x[lo:hi].T
        in_maps.append({
            "xT": xs, "W1e": W1e, "W2e": W2e, "W3e": W3e,
            "b1": b1, "b2": b2, "b3": b3,
            "epack": pre["epack"][c], "epack3": pre["epack3"][c],
        })
    return in_maps


_CACHE = {}


def _gather_lanes(nc):
    """issue-index -> DMASW lane (0..7), by walking InstDMAGatherAnt in
    issue order (instruction id order)."""
    insts = []
    for bb in nc.m.functions[0].blocks:
        for inst in bb.instructions:
            if type(inst).__name__ == "InstDMAGatherAnt":
                insts.append(inst)
    insts.sort(key=lambda i: int(i.name.split("-")[1]))
    lanes = []
    for inst in insts:
        proc = getattr(inst, "bass_scheduled_proc", None)
        lanes.append(None if proc is None else proc - 11)  # DMASW0 == 11
    return lanes


def get_program(cfg: Cfg, TA, TB, T3A, T3B, repeat=1):
    key = (cfg.n_nodes, TA, TB, T3A, T3B, repeat, CH, QSPREAD, ACTSWA,
           NEGPAD)
    if key not in _CACHE:
        nc = build_program(cfg, TA, TB, T3A, T3B, repeat)
        if QSPREAD:
            lanes = _gather_lanes(nc)
            if all(ln is not None and 0 <= ln < 8 for ln in lanes):
                qmap = [ln % 4 for ln in lanes]
                nc2 = build_program(cfg, TA, TB, T3A, T3B, repeat,
                                    queue_map=qmap)
                # verify the schedule landed the same way
                lanes2 = _gather_lanes(nc2)
                if lanes2 == lanes:
                    nc = nc2
        _CACHE[key] = nc
    return _CACHE[key]


def run(inputs, cfg: Cfg, trace=False):
    pre = preprocess(inputs["edge_src"], inputs["edge_dst"], cfg)
    in_maps = make_in_maps(inputs, pre, cfg)
    nc = get_program(cfg, pre["TA"], pre["TB"], pre["T3A"], pre["T3B"])
    res = run_bass_kernel_spmd(nc, in_maps, list(range(NCORES)), trace=trace)
    outs = []
    for c in range(NCORES):
        outs.append(res.results[c]["out"][0:pre["zcounts"][c], :])
    return np.concatenate(outs, axis=0).astype(np.float32), res


def kernel(**inputs):
    cfg = Cfg(n_nodes=inputs["x"].shape[0],
              in_ch=inputs["x"].shape[1],
              hid=inputs["W1"].shape[1],
              out_ch=inputs["W3"].shape[1])
    out, _ = run(inputs, cfg)
    return out
